# revision 57
# baseline (speedup 1.0000x reference)
"""GAT layer (nn_GATLayer) on 8 Trainium2 NeuronCores via Bass/Tile.

Strategy (dst-partitioned, diagonal-packed blocks, softmax fully local):
  - Core k owns dst nodes [k*6250, (k+1)*6250). All in-edges of those nodes are
    processed on core k, so segment-softmax needs no cross-core reduction.
  - Phase A (replicated): z = h @ W.T for ALL nodes with the attention
    projections folded in: rhs = [W.T | W.T@A1blk | W.T@A2blk] giving per-node
    rows [z(64) | as(4) | ad(4)] in one matmul (bf16 in, f32 PSUM). Node rows
    are packed PAIRWISE into 512B HBM rows: pair r = nodes (a, a+128) with
    a = (r//128)*256 + r%128, each half [z bf16 x64 | as f32 x4 | 112B pad].
    512B rows keep gather indices < 32768 (int16 SWDGE limit) with no
    small-transfer DMA penalty, at the cost of a per-edge half-select.
  - Phase A0: same matmul over own nodes only, in block-packed column order;
    ad(4) per own node stays SBUF-resident as adst[128 pos, 49 block, 4].
  - Edge phase: per core, dst nodes are sorted by in-degree (desc) and cut
    into 49 blocks of 128. Block b uses K_b = max in-degree in block chunks;
    slot (p, c) holds the c-th in-edge of the node at position p (pads point
    at a poison pair row whose both halves have as = -87 => exp ~ 0).
    Per block: dma_gather 512B pair rows by src; half-select the 144B node
    payload (Act copy of the hi half + DVE copy_predicated of the lo half);
    e = as + ad[p] folds into the Act exp bias (per-head calls);
    ex = max(exp(e), exp(0.2 e)) (leaky-relu folded, softmax shift-free);
    num = sum_c (z_sel * ex) via bf16 mult + f32 tensor_reduce; den likewise.
    Block emission is software-pipelined (gather/select/exps/rest staggered
    across iterations) so no engine queue head-of-line blocks another block's
    ready work. One final reciprocal + multiply normalizes all blocks; outO
    row = p*49 + b and the host permutes rows back to node order.
"""

import numpy as np

N_NODES = 50000
N_EDGES = 800000
IN_FEATS = 128
OUT_FEATS = 16
NUM_HEADS = 4
ALPHA = 0.2
HF = NUM_HEADS * OUT_FEATS  # 64

N_CORES = 8
P = 128
NODES_PER_CORE = N_NODES // N_CORES     # 6250
BLOCKS = 49                              # ceil(6250/128) blocks of 128 dst nodes
NODE_PAD = BLOCKS * P                    # 6272
N_NODES_PAD = 50176                      # 392 * 128 (node chunks in phase A)
NPAIR = N_NODES_PAD // 2                 # 25088 pair rows
POISON = NPAIR                           # poison pair row index
CMAX = 8                                 # chunks (1024 idxs) per dma_gather call


def _pair_row(n):
    return (n // 256) * 128 + (n % 128)


def _pair_half(n):
    return (n // 128) & 1


def _wrap16(vals):
    # stream position i -> idx tile [16, n/16] at [i%16, i//16]; rows
    # replicated to 128 partitions (SWDGE reads a [128, n/16] view).
    n = vals.shape[0]
    w = vals.reshape(n // 16, 16).T                    # [16, n/16]
    return np.ascontiguousarray(np.tile(w, (8, 1)))   # [128, n/16]


def _build_host_data(h, src, dst, W, A):
    """All index/layout prep. Returns per-core input dicts + block chunk counts."""
    src = np.asarray(src)
    dst = np.asarray(dst)
    W = np.asarray(W, dtype=np.float32)
    A = np.asarray(A, dtype=np.float32)
    h = np.asarray(h, dtype=np.float32)
    import ml_dtypes

    # folded weights: [W.T | W.T@A1blk | W.T@A2blk]  -> [128, 72]
    A1blk = np.zeros((HF, NUM_HEADS), dtype=np.float32)
    A2blk = np.zeros((HF, NUM_HEADS), dtype=np.float32)
    for hd in range(NUM_HEADS):
        A1blk[hd * OUT_FEATS:(hd + 1) * OUT_FEATS, hd] = A[hd, :OUT_FEATS]
        A2blk[hd * OUT_FEATS:(hd + 1) * OUT_FEATS, hd] = A[hd, OUT_FEATS:]
    WT = np.ascontiguousarray(W.T)                       # [128, 64]
    wcat = np.concatenate([WT, WT @ A1blk, WT @ A2blk], axis=1)  # [128, 72]
    wcat = wcat.astype(ml_dtypes.bfloat16)

    hT = np.zeros((P, N_NODES_PAD), dtype=ml_dtypes.bfloat16)    # [128, 50176]
    hT[:, :N_NODES] = h.T.astype(ml_dtypes.bfloat16)

    # poison pair row: both halves z=0, as=-87 -> ex ~ exp(-17.4) == ~0
    # but nonzero, so isolated nodes still get a finite denominator
    poison = np.zeros((1, 128), dtype=np.float32)
    poison[0, 32:36] = -87.0
    poison[0, 96:100] = -87.0

    # per-core edge runs grouped by dst
    order = np.argsort(dst, kind="stable")
    dst_s = dst[order]
    src_s = src[order]
    core_begin = np.searchsorted(dst_s, np.arange(0, N_NODES + 1, NODES_PER_CORE))

    # degree-sorted block assignment; K_b must be uniform across cores (SPMD)
    per_core = []
    kbs_cores = []
    for k in range(N_CORES):
        lo_e, hi_e = core_begin[k], core_begin[k + 1]
        cs = src_s[lo_e:hi_e]
        cd = dst_s[lo_e:hi_e] - k * NODES_PER_CORE      # local dst [0, 6250)
        deg = np.bincount(cd, minlength=NODES_PER_CORE)
        node_order = np.argsort(-deg, kind="stable")    # block = rank // 128
        degs = np.zeros(NODE_PAD, dtype=np.int64)
        degs[:NODES_PER_CORE] = deg[node_order]
        kbs = degs.reshape(BLOCKS, P).max(axis=1)
        kbs_cores.append(np.maximum(kbs, 1))
        # edge run start per local node (dst-sorted, stable)
        starts = np.zeros(NODES_PER_CORE + 1, dtype=np.int64)
        np.cumsum(deg, out=starts[1:])
        per_core.append((cs, node_order, deg, starts))

    KBS = tuple(int(x) for x in np.maximum.reduce(kbs_cores))
    SUMK = sum(KBS)
    S = P * SUMK                    # total edge slots per core
    offs = np.concatenate([[0], np.cumsum(KBS)]).astype(np.int64)

    in_maps = []
    unpack_maps = []
    for k in range(N_CORES):
        cs, node_order, deg, starts = per_core[k]
        # node -> (block, pos)
        blk = np.empty(NODES_PER_CORE, dtype=np.int64)
        pos = np.empty(NODES_PER_CORE, dtype=np.int64)
        ranks = np.empty(NODES_PER_CORE, dtype=np.int64)
        ranks[node_order] = np.arange(NODES_PER_CORE)
        blk = ranks // P
        pos = ranks % P
        unpack_maps.append(pos * BLOCKS + blk)          # node -> outO row

        gidx = np.full(S, POISON, dtype=np.int16)
        par = np.zeros((P, SUMK), dtype=np.int8)         # 1 -> take LO half
        cd = dst_s[core_begin[k]:core_begin[k + 1]] - k * NODES_PER_CORE
        ordinal = np.arange(len(cd)) - starts[cd]        # c-th in-edge of node
        offs_arr = offs[:-1]
        ci = offs_arr[blk[cd]] + ordinal                 # chunk column in tables
        gidx[ci * P + pos[cd]] = ((cs // 256) * 128 + (cs % 128)).astype(np.int16)
        par.reshape(-1)[pos[cd] * SUMK + ci] = ((cs // 128) & 1 == 0).astype(np.int8)

        # column b*128+pos = h of that node (block-packed own-node order)
        hT_own = np.zeros((P, NODE_PAD), dtype=ml_dtypes.bfloat16)
        cols = blk * P + pos
        hT_own[:, cols] = hT[:, np.arange(NODES_PER_CORE) + k * NODES_PER_CORE]

        in_maps.append({
            "hT": hT,
            "hT_own": hT_own,
            "wcat": wcat,
            "gidx": _wrap16(gidx),
            "par": np.ascontiguousarray(par),
            "poison": poison,
        })
    return in_maps, KBS, 0, unpack_maps


def _build_program(KBS, _unused=0):
    import concourse.bacc as bacc
    import concourse.tile as tile
    import concourse.mybir as mybir

    SUMK = sum(KBS)
    S = P * SUMK
    KMAX = max(KBS)
    offs = [0]
    for kb in KBS:
        offs.append(offs[-1] + kb)
    f32 = mybir.dt.float32
    bf16 = mybir.dt.bfloat16
    i16 = mybir.dt.int16
    i32 = mybir.dt.int32

    nc = bacc.Bacc("TRN2", target_bir_lowering=False, debug=False)

    hT = nc.dram_tensor("hT", [P, N_NODES_PAD], bf16, kind="ExternalInput")
    hT_own = nc.dram_tensor("hT_own", [P, NODE_PAD], bf16, kind="ExternalInput")
    wcat_d = nc.dram_tensor("wcat", [P, 72], bf16, kind="ExternalInput")
    gidx_d = nc.dram_tensor("gidx", [P, S // 16], i16, kind="ExternalInput")
    par_d = nc.dram_tensor("par", [P, SUMK], mybir.dt.int8, kind="ExternalInput")
    poison_d = nc.dram_tensor("poison", [1, 128], f32, kind="ExternalInput")

    zaug = nc.dram_tensor("zaug", [NPAIR + 1, 128], f32)  # 512B pair rows + poison
    outO = nc.dram_tensor("outO", [P * BLOCKS, HF], f32, kind="ExternalOutput")

    NCHUNK = N_NODES_PAD // P        # 392
    SC = 4                           # chunks per PSUM tile
    SC2 = 2 * SC                     # chunks per load/store super-chunk

    with tile.TileContext(nc) as tc:
        with (
            tc.tile_pool(name="const", bufs=1) as cpool,
            tc.tile_pool(name="pa", bufs=6) as pa,
            tc.tile_pool(name="papsum", bufs=4, space="PSUM") as papsum,
            tc.tile_pool(name="edge", bufs=4) as ep,
            tc.tile_pool(name="flush", bufs=3) as fp,
        ):
            wcat_t = cpool.tile([P, 72], bf16)
            nc.sync.dma_start(out=wcat_t[:], in_=wcat_d[:])
            gidx_t = cpool.tile([P, S // 16], i16)
            nc.sync.dma_start(out=gidx_t[:], in_=gidx_d[:])
            par_t = cpool.tile([P, SUMK], mybir.dt.int8)
            nc.sync.dma_start(out=par_t[:], in_=par_d[:])
            # poison row -> zaug[NPAIR]
            poi_t = cpool.tile([1, 128], f32)
            nc.sync.dma_start(out=poi_t[:], in_=poison_d[:])
            nc.sync.dma_start(out=zaug[NPAIR:NPAIR + 1, :], in_=poi_t[:])

            # ---------------- Phase A0: ad for own nodes (SBUF-resident) -----
            # (groups emitted interleaved into the phase A loop below)
            adst = cpool.tile([P, BLOCKS, 4], f32)
            hown_t = cpool.tile([P, NODE_PAD], bf16)
            nc.sync.dma_start(out=hown_t[:], in_=hT_own[:])

            def emit_a0_group(c0):
                c1 = min(c0 + SC, BLOCKS)
                zp = papsum.tile([P, SC, 72], f32, tag="zp")
                for j in range(c0, c1):
                    nc.tensor.matmul(
                        out=zp[:, j - c0, :],
                        lhsT=hown_t[:, j * P:(j + 1) * P],
                        rhs=wcat_t[:],
                        start=True, stop=True,
                    )
                nc.vector.tensor_copy(adst[:, c0:c1, :], zp[:, :c1 - c0, 68:72])

            # ---------------- Phase A: zaug pair rows for all nodes ----------
            n_sc = NCHUNK // SC2                        # 49, exact
            n_a0 = (BLOCKS + SC - 1) // SC              # 13 A0 groups
            z3 = zaug[0:NPAIR, :].rearrange("(c2 q) (c01 e) -> q c2 c01 e",
                                            q=P, c01=2)
            for sidx in range(n_sc):
                if sidx < n_a0:
                    emit_a0_group(sidx * SC)
                base = sidx * SC2 * P
                hsl = pa.tile([P, SC2 * P], bf16, tag="hsl")
                nc.sync.dma_start(out=hsl[:], in_=hT[:, base:base + SC2 * P])
                zst = pa.tile([P, SC2, 64], f32, tag="zst")
                zbf = zst[:].bitcast(mybir.dt.bfloat16)
                for g0 in range(0, SC2, SC):
                    g1 = g0 + SC
                    zp = papsum.tile([P, SC, 72], f32, tag="zp")
                    for j in range(g0, g1):
                        nc.tensor.matmul(
                            out=zp[:, j - g0, :],
                            lhsT=hsl[:, j * P:(j + 1) * P],
                            rhs=wcat_t[:],
                            start=True, stop=True,
                        )
                    nc.scalar.copy(out=zbf[:, g0:g1, 0:64], in_=zp[:, :SC, 0:64])
                    nc.vector.tensor_copy(zst[:, g0:g1, 32:36], zp[:, :SC, 64:68])
                zs4 = zst[:].rearrange("p (c2 c01) e -> p c2 c01 e", c01=2)
                nc.sync.dma_start(
                    out=z3[:, sidx * SC2 // 2:(sidx + 1) * SC2 // 2, :, :],
                    in_=zs4[:],
                )
            adst02 = cpool.tile([P, BLOCKS, 4], f32)
            nc.vector.tensor_scalar(out=adst02[:], in0=adst[:], scalar1=ALPHA,
                                    scalar2=None, op0=mybir.AluOpType.mult)

            # ---------------- Edge phase ----------------
            # Software-pipelined issue order: emitting gather(b), select(b-LS),
            # exps(b-LE), rest(b-LR) per iteration keeps every engine queue
            # free of head-of-line blocking (in-order queues would otherwise
            # serialize each block's Act<->DVE ping-pong).
            # [num(64) | den(4)] per block, split into two tiles so the first
            # group's normalization (whole-tile dependency) can issue while
            # later blocks still compute
            GSPLIT = 46
            outd1 = cpool.tile([P, GSPLIT, 68], f32)
            outd2 = cpool.tile([P, BLOCKS - GSPLIT, 68], f32)
            outb = cpool.tile([P, GSPLIT, HF], f32)   # reused by both norm groups
            zs_t = {}
            zsel_t = {}
            ex1_t = {}
            rhs_t = {}

            def emit_gather(b):
                K = KBS[b]
                off = offs[b]
                zs = ep.tile([P, KMAX, 128], f32, tag="zs", name=f"zs{b}")
                zs_t[b] = zs
                for c0 in range(0, K, CMAX):
                    c1 = min(c0 + CMAX, K)
                    nc.gpsimd.dma_gather(
                        out_ap=zs[:, c0:c1, :],
                        in_ap=zaug[:],
                        idxs_ap=gidx_t[:, (off + c0) * 8:(off + c1) * 8],
                        num_idxs=(c1 - c0) * P,
                        num_idxs_reg=(c1 - c0) * P,
                        elem_size=128,
                    )

            def emit_select(b):
                # half-select the 144B payload [z bf16 x64 | as f32 x4]:
                # hi-half copied on Act (z words are never NaN/denormal here),
                # then lo words overwrite where par!=0 (DVE predicated copy)
                K = KBS[b]
                off = offs[b]
                zs = zs_t.pop(b)
                zs_i = zs[:].bitcast(i32)
                zsel = ep.tile([P, KMAX, 36], f32, tag="zsel", name=f"zsel{b}")
                zsel_t[b] = zsel
                nc.scalar.copy(out=zsel[:, :K, :], in_=zs[:, :K, 64:100])
                parm = par_t[:, off:off + K].unsqueeze(2).broadcast_to([P, K, 36])
                nc.vector.copy_predicated(zsel[:].bitcast(i32)[:, :K, :], parm,
                                          zs_i[:, :K, 0:36])

            def emit_exps(b):
                # e = as + ad[p]; ex = exp(leaky(e)) = max(exp(e), exp(.2e));
                # the +ad folds into the Act bias (per-head calls)
                K = KBS[b]
                zsel = zsel_t[b]
                ex1 = ep.tile([P, KMAX, 4], f32, tag="ex1", name=f"ex1{b}")
                rhs = ep.tile([P, KMAX, 36], f32, tag="rhs", name=f"rhs{b}")
                ex1_t[b] = ex1
                rhs_t[b] = rhs
                rhs_bf = rhs[:].bitcast(mybir.dt.bfloat16)
                for hh in range(4):
                    nc.scalar.activation(ex1[:, :K, hh], zsel[:, :K, 32 + hh],
                                         mybir.ActivationFunctionType.Exp,
                                         bias=adst[:, b, hh:hh + 1])
                    nc.scalar.activation(rhs_bf[:, :K, 64 + hh], zsel[:, :K, 32 + hh],
                                         mybir.ActivationFunctionType.Exp,
                                         scale=ALPHA,
                                         bias=adst02[:, b, hh:hh + 1])

            def emit_rest(b):
                K = KBS[b]
                zsel = zsel_t.pop(b)
                ex1 = ex1_t.pop(b)
                rhs = rhs_t.pop(b)
                rhs_bf = rhs[:].bitcast(mybir.dt.bfloat16)
                nc.vector.tensor_tensor(out=rhs_bf[:, :K, 64:68],
                                        in0=rhs_bf[:, :K, 64:68],
                                        in1=ex1[:, :K, :], op=mybir.AluOpType.max)
                # num terms in bf16 occupy rhs words 0:32: t1 = z_sel * ex
                zsb = zsel[:].bitcast(mybir.dt.bfloat16)[:, :K, 0:64].rearrange(
                    "p k (h f) -> p k h f", h=4)
                exb = rhs_bf[:, :K, 64:68].unsqueeze(3).broadcast_to([P, K, 4, 16])
                t4 = rhs_bf[:, :K, 0:64].rearrange("p k (h f) -> p k h f", h=4)
                nc.vector.tensor_tensor(out=t4, in0=zsb, in1=exb,
                                        op=mybir.AluOpType.mult)
                # one fused reduce over chunks: [num x64 | ex x4] bf16 -> f32
                outd, bb = (outd1, b) if b < GSPLIT else (outd2, b - GSPLIT)
                nc.vector.tensor_reduce(
                    out=outd[:, bb, 0:68],
                    in_=rhs_bf[:, :K, 0:68].rearrange("p k f -> p f k"),
                    axis=mybir.AxisListType.X, op=mybir.AluOpType.add)

            recb = cpool.tile([P, BLOCKS, 4], f32)
            o3 = outO[:].rearrange("(p b) e -> p b e", p=P)

            def emit_norm(outd, b0, b1):
                # normalize + store blocks [b0, b1); outd holds exactly this
                # group so the dependency on its reduces is whole-tile
                nb = b1 - b0
                nc.vector.reciprocal(recb[:, b0:b1, :], outd[:, :, 64:68])
                rb = recb[:, b0:b1, :].unsqueeze(3).broadcast_to([P, nb, 4, 16])
                o4 = outb[:, 0:nb, :].rearrange("p b (h f) -> p b h f", h=4)
                nc.vector.tensor_tensor(
                    out=o4,
                    in0=outd[:, :, 0:64].rearrange("p b (h f) -> p b h f", h=4),
                    in1=rb, op=mybir.AluOpType.mult)
                nc.sync.dma_start(out=o3[:, b0:b1, :], in_=outb[:, 0:nb, :])

            LS, LE, LR = 1, 2, 3
            for i in range(BLOCKS + LR):
                if i < BLOCKS:
                    emit_gather(i)
                if 0 <= i - LS < BLOCKS:
                    emit_select(i - LS)
                if 0 <= i - LE < BLOCKS:
                    emit_exps(i - LE)
                if 0 <= i - LR < BLOCKS:
                    emit_rest(i - LR)
                if i - LR == GSPLIT - 1:
                    emit_norm(outd1, 0, GSPLIT)
            emit_norm(outd2, GSPLIT, BLOCKS)

    nc.finalize()
    return nc


def kernel(h, src, dst, W, A):
    from concourse.bass_utils import run_bass_kernel_spmd

    in_maps, KBS, _, unpack_maps = _build_host_data(h, src, dst, W, A)
    nc = _build_program(KBS)
    res = run_bass_kernel_spmd(nc, in_maps, core_ids=list(range(N_CORES)))
    out = np.empty((N_NODES, HF), dtype=np.float32)
    for k in range(N_CORES):
        out[k * NODES_PER_CORE:(k + 1) * NODES_PER_CORE] = \
            res.results[k]["outO"][unpack_maps[k]]
    return out


# revision 58
# speedup vs baseline: 1.0115x; 1.0115x over previous
"""GAT layer (nn_GATLayer) on 8 Trainium2 NeuronCores via Bass/Tile.

Strategy (dst-partitioned, diagonal-packed blocks, softmax fully local):
  - Core k owns dst nodes [k*6250, (k+1)*6250). All in-edges of those nodes are
    processed on core k, so segment-softmax needs no cross-core reduction.
  - Phase A (replicated): z = h @ W.T for ALL nodes with the attention
    projections folded in: rhs = [W.T | W.T@A1blk | W.T@A2blk] giving per-node
    rows [z(64) | as(4) | ad(4)] in one matmul (bf16 in, f32 PSUM). Node rows
    are packed PAIRWISE into 512B HBM rows: pair r = nodes (a, a+128) with
    a = (r//128)*256 + r%128, each half [z bf16 x64 | as f32 x4 | 112B pad].
    512B rows keep gather indices < 32768 (int16 SWDGE limit) with no
    small-transfer DMA penalty, at the cost of a per-edge half-select.
  - Phase A0: same matmul over own nodes only, in block-packed column order;
    ad(4) per own node stays SBUF-resident as adst[128 pos, 49 block, 4].
  - Edge phase: per core, dst nodes are sorted by in-degree (desc) and cut
    into 49 blocks of 128. Block b uses K_b = max in-degree in block chunks;
    slot (p, c) holds the c-th in-edge of the node at position p (pads point
    at a poison pair row whose both halves have as = -87 => exp ~ 0).
    Per block: dma_gather 512B pair rows by src; half-select the 144B node
    payload (Act copy of the hi half + DVE copy_predicated of the lo half);
    e = as + ad[p] folds into the Act exp bias (per-head calls);
    ex = max(exp(e), exp(0.2 e)) (leaky-relu folded, softmax shift-free);
    num = sum_c (z_sel * ex) via bf16 mult + f32 tensor_reduce; den likewise.
    Block emission is software-pipelined (gather/select/exps/rest staggered
    across iterations) so no engine queue head-of-line blocks another block's
    ready work. One final reciprocal + multiply normalizes all blocks; outO
    row = p*49 + b and the host permutes rows back to node order.
"""

import numpy as np

N_NODES = 50000
N_EDGES = 800000
IN_FEATS = 128
OUT_FEATS = 16
NUM_HEADS = 4
ALPHA = 0.2
HF = NUM_HEADS * OUT_FEATS  # 64

N_CORES = 8
P = 128
NODES_PER_CORE = N_NODES // N_CORES     # 6250
BLOCKS = 49                              # ceil(6250/128) blocks of 128 dst nodes
NODE_PAD = BLOCKS * P                    # 6272
N_NODES_PAD = 50176                      # 392 * 128 (node chunks in phase A)
NPAIR = N_NODES_PAD // 2                 # 25088 pair rows
POISON = NPAIR                           # poison pair row index
CMAX = 8                                 # chunks (1024 idxs) per dma_gather call


def _pair_row(n):
    return (n // 256) * 128 + (n % 128)


def _pair_half(n):
    return (n // 128) & 1


def _wrap16(vals):
    # stream position i -> idx tile [16, n/16] at [i%16, i//16]; rows
    # replicated to 128 partitions (SWDGE reads a [128, n/16] view).
    n = vals.shape[0]
    w = vals.reshape(n // 16, 16).T                    # [16, n/16]
    return np.ascontiguousarray(np.tile(w, (8, 1)))   # [128, n/16]


def _build_host_data(h, src, dst, W, A):
    """All index/layout prep. Returns per-core input dicts + block chunk counts."""
    src = np.asarray(src)
    dst = np.asarray(dst)
    W = np.asarray(W, dtype=np.float32)
    A = np.asarray(A, dtype=np.float32)
    h = np.asarray(h, dtype=np.float32)
    import ml_dtypes

    # folded weights: [W.T | W.T@A1blk | W.T@A2blk]  -> [128, 72]
    A1blk = np.zeros((HF, NUM_HEADS), dtype=np.float32)
    A2blk = np.zeros((HF, NUM_HEADS), dtype=np.float32)
    for hd in range(NUM_HEADS):
        A1blk[hd * OUT_FEATS:(hd + 1) * OUT_FEATS, hd] = A[hd, :OUT_FEATS]
        A2blk[hd * OUT_FEATS:(hd + 1) * OUT_FEATS, hd] = A[hd, OUT_FEATS:]
    WT = np.ascontiguousarray(W.T)                       # [128, 64]
    wcat = np.concatenate([WT, WT @ A1blk, WT @ A2blk], axis=1)  # [128, 72]
    wcat = wcat.astype(ml_dtypes.bfloat16)

    hT = np.zeros((P, N_NODES_PAD), dtype=ml_dtypes.bfloat16)    # [128, 50176]
    hT[:, :N_NODES] = h.T.astype(ml_dtypes.bfloat16)

    # poison pair row: both halves z=0, as=-87 -> ex ~ exp(-17.4) == ~0
    # but nonzero, so isolated nodes still get a finite denominator
    poison = np.zeros((1, 128), dtype=np.float32)
    poison[0, 32:36] = -87.0
    poison[0, 96:100] = -87.0

    # per-core edge runs grouped by dst
    order = np.argsort(dst, kind="stable")
    dst_s = dst[order]
    src_s = src[order]
    core_begin = np.searchsorted(dst_s, np.arange(0, N_NODES + 1, NODES_PER_CORE))

    # degree-sorted block assignment; K_b must be uniform across cores (SPMD)
    per_core = []
    kbs_cores = []
    for k in range(N_CORES):
        lo_e, hi_e = core_begin[k], core_begin[k + 1]
        cs = src_s[lo_e:hi_e]
        cd = dst_s[lo_e:hi_e] - k * NODES_PER_CORE      # local dst [0, 6250)
        deg = np.bincount(cd, minlength=NODES_PER_CORE)
        node_order = np.argsort(-deg, kind="stable")    # block = rank // 128
        degs = np.zeros(NODE_PAD, dtype=np.int64)
        degs[:NODES_PER_CORE] = deg[node_order]
        kbs = degs.reshape(BLOCKS, P).max(axis=1)
        kbs_cores.append(np.maximum(kbs, 1))
        # edge run start per local node (dst-sorted, stable)
        starts = np.zeros(NODES_PER_CORE + 1, dtype=np.int64)
        np.cumsum(deg, out=starts[1:])
        per_core.append((cs, node_order, deg, starts))

    KBS = tuple(int(x) for x in np.maximum.reduce(kbs_cores))
    SUMK = sum(KBS)
    S = P * SUMK                    # total edge slots per core
    offs = np.concatenate([[0], np.cumsum(KBS)]).astype(np.int64)

    in_maps = []
    unpack_maps = []
    for k in range(N_CORES):
        cs, node_order, deg, starts = per_core[k]
        # node -> (block, pos)
        blk = np.empty(NODES_PER_CORE, dtype=np.int64)
        pos = np.empty(NODES_PER_CORE, dtype=np.int64)
        ranks = np.empty(NODES_PER_CORE, dtype=np.int64)
        ranks[node_order] = np.arange(NODES_PER_CORE)
        blk = ranks // P
        pos = ranks % P
        unpack_maps.append(pos * BLOCKS + blk)          # node -> outO row

        gidx = np.full(S, POISON, dtype=np.int16)
        par = np.zeros((P, SUMK), dtype=np.int8)         # 1 -> take LO half
        cd = dst_s[core_begin[k]:core_begin[k + 1]] - k * NODES_PER_CORE
        ordinal = np.arange(len(cd)) - starts[cd]        # c-th in-edge of node
        offs_arr = offs[:-1]
        ci = offs_arr[blk[cd]] + ordinal                 # chunk column in tables
        gidx[ci * P + pos[cd]] = ((cs // 256) * 128 + (cs % 128)).astype(np.int16)
        par.reshape(-1)[pos[cd] * SUMK + ci] = ((cs // 128) & 1 == 0).astype(np.int8)

        # column b*128+pos = h of that node (block-packed own-node order)
        hT_own = np.zeros((P, NODE_PAD), dtype=ml_dtypes.bfloat16)
        cols = blk * P + pos
        hT_own[:, cols] = hT[:, np.arange(NODES_PER_CORE) + k * NODES_PER_CORE]

        in_maps.append({
            "hT": hT,
            "hT_own": hT_own,
            "wcat": wcat,
            "gidx": _wrap16(gidx),
            "par": np.ascontiguousarray(par),
            "poison": poison,
        })
    return in_maps, KBS, 0, unpack_maps


def _build_program(KBS, _unused=0):
    import concourse.bacc as bacc
    import concourse.tile as tile
    import concourse.mybir as mybir

    SUMK = sum(KBS)
    S = P * SUMK
    KMAX = max(KBS)
    offs = [0]
    for kb in KBS:
        offs.append(offs[-1] + kb)
    f32 = mybir.dt.float32
    bf16 = mybir.dt.bfloat16
    i16 = mybir.dt.int16
    i32 = mybir.dt.int32

    nc = bacc.Bacc("TRN2", target_bir_lowering=False, debug=False)

    hT = nc.dram_tensor("hT", [P, N_NODES_PAD], bf16, kind="ExternalInput")
    hT_own = nc.dram_tensor("hT_own", [P, NODE_PAD], bf16, kind="ExternalInput")
    wcat_d = nc.dram_tensor("wcat", [P, 72], bf16, kind="ExternalInput")
    gidx_d = nc.dram_tensor("gidx", [P, S // 16], i16, kind="ExternalInput")
    par_d = nc.dram_tensor("par", [P, SUMK], mybir.dt.int8, kind="ExternalInput")
    poison_d = nc.dram_tensor("poison", [1, 128], f32, kind="ExternalInput")

    zaug = nc.dram_tensor("zaug", [NPAIR + 1, 128], f32)  # 512B pair rows + poison
    outO = nc.dram_tensor("outO", [P * BLOCKS, HF], f32, kind="ExternalOutput")

    NCHUNK = N_NODES_PAD // P        # 392
    SC = 4                           # chunks per PSUM tile
    SC2 = 2 * SC                     # chunks per load/store super-chunk

    with tile.TileContext(nc) as tc:
        with (
            tc.tile_pool(name="const", bufs=1) as cpool,
            tc.tile_pool(name="pa", bufs=6) as pa,
            tc.tile_pool(name="papsum", bufs=4, space="PSUM") as papsum,
            tc.tile_pool(name="edge", bufs=4) as ep,
            tc.tile_pool(name="flush", bufs=3) as fp,
        ):
            wcat_t = cpool.tile([P, 72], bf16)
            nc.sync.dma_start(out=wcat_t[:], in_=wcat_d[:])
            gidx_t = cpool.tile([P, S // 16], i16)
            nc.sync.dma_start(out=gidx_t[:], in_=gidx_d[:])
            par_t = cpool.tile([P, SUMK], mybir.dt.int8)
            nc.sync.dma_start(out=par_t[:], in_=par_d[:])
            # poison row -> zaug[NPAIR]
            poi_t = cpool.tile([1, 128], f32)
            nc.sync.dma_start(out=poi_t[:], in_=poison_d[:])
            nc.sync.dma_start(out=zaug[NPAIR:NPAIR + 1, :], in_=poi_t[:])

            # ---------------- Phase A0: ad for own nodes (SBUF-resident) -----
            # (groups emitted interleaved into the phase A loop below)
            adst = cpool.tile([P, BLOCKS, 4], f32)
            hown_t = cpool.tile([P, NODE_PAD], bf16)
            nc.sync.dma_start(out=hown_t[:], in_=hT_own[:])

            def emit_a0_group(c0):
                c1 = min(c0 + SC, BLOCKS)
                zp = papsum.tile([P, SC, 72], f32, tag="zp")
                for j in range(c0, c1):
                    nc.tensor.matmul(
                        out=zp[:, j - c0, :],
                        lhsT=hown_t[:, j * P:(j + 1) * P],
                        rhs=wcat_t[:],
                        start=True, stop=True,
                    )
                nc.vector.tensor_copy(adst[:, c0:c1, :], zp[:, :c1 - c0, 68:72])

            # ---------------- Phase A: zaug pair rows for all nodes ----------
            n_sc = NCHUNK // SC2                        # 49, exact
            n_a0 = (BLOCKS + SC - 1) // SC              # 13 A0 groups
            z3 = zaug[0:NPAIR, :].rearrange("(c2 q) (c01 e) -> q c2 c01 e",
                                            q=P, c01=2)
            for sidx in range(n_sc):
                if sidx < n_a0:
                    emit_a0_group(sidx * SC)
                base = sidx * SC2 * P
                hsl = pa.tile([P, SC2 * P], bf16, tag="hsl")
                nc.sync.dma_start(out=hsl[:], in_=hT[:, base:base + SC2 * P])
                zst = pa.tile([P, SC2, 64], f32, tag="zst")
                zbf = zst[:].bitcast(mybir.dt.bfloat16)
                for g0 in range(0, SC2, SC):
                    g1 = g0 + SC
                    zp = papsum.tile([P, SC, 72], f32, tag="zp")
                    for j in range(g0, g1):
                        nc.tensor.matmul(
                            out=zp[:, j - g0, :],
                            lhsT=hsl[:, j * P:(j + 1) * P],
                            rhs=wcat_t[:],
                            start=True, stop=True,
                        )
                    nc.scalar.copy(out=zbf[:, g0:g1, 0:64], in_=zp[:, :SC, 0:64])
                    nc.vector.tensor_copy(zst[:, g0:g1, 32:36], zp[:, :SC, 64:68])
                zs4 = zst[:].rearrange("p (c2 c01) e -> p c2 c01 e", c01=2)
                nc.sync.dma_start(
                    out=z3[:, sidx * SC2 // 2:(sidx + 1) * SC2 // 2, :, :],
                    in_=zs4[:],
                )
            adst02 = cpool.tile([P, BLOCKS, 4], f32)
            nc.vector.tensor_scalar(out=adst02[:], in0=adst[:], scalar1=ALPHA,
                                    scalar2=None, op0=mybir.AluOpType.mult)

            # ---------------- Edge phase ----------------
            # Software-pipelined issue order: emitting gather(b), select(b-LS),
            # exps(b-LE), rest(b-LR) per iteration keeps every engine queue
            # free of head-of-line blocking (in-order queues would otherwise
            # serialize each block's Act<->DVE ping-pong).
            # [num(64) | den(4)] per block, split into two tiles so the first
            # group's normalization (whole-tile dependency) can issue while
            # later blocks still compute
            GSPLIT = 42
            outd1 = cpool.tile([P, GSPLIT, 68], f32)
            outd2 = cpool.tile([P, BLOCKS - GSPLIT, 68], f32)
            outb = cpool.tile([P, GSPLIT, HF], f32)   # reused by both norm groups
            zs_t = {}
            zsel_t = {}
            ex1_t = {}
            rhs_t = {}

            def emit_gather(b):
                K = KBS[b]
                off = offs[b]
                zs = ep.tile([P, KMAX, 128], f32, tag="zs", name=f"zs{b}")
                zs_t[b] = zs
                for c0 in range(0, K, CMAX):
                    c1 = min(c0 + CMAX, K)
                    nc.gpsimd.dma_gather(
                        out_ap=zs[:, c0:c1, :],
                        in_ap=zaug[:],
                        idxs_ap=gidx_t[:, (off + c0) * 8:(off + c1) * 8],
                        num_idxs=(c1 - c0) * P,
                        num_idxs_reg=(c1 - c0) * P,
                        elem_size=128,
                    )

            def emit_select(b):
                # half-select the 144B payload [z bf16 x64 | as f32 x4]:
                # hi-half copied on Act (z words are never NaN/denormal here),
                # then lo words overwrite where par!=0 (DVE predicated copy)
                K = KBS[b]
                off = offs[b]
                zs = zs_t.pop(b)
                zs_i = zs[:].bitcast(i32)
                zsel = ep.tile([P, KMAX, 36], f32, tag="zsel", name=f"zsel{b}")
                zsel_t[b] = zsel
                nc.scalar.copy(out=zsel[:, :K, :], in_=zs[:, :K, 64:100])
                parm = par_t[:, off:off + K].unsqueeze(2).broadcast_to([P, K, 36])
                nc.vector.copy_predicated(zsel[:].bitcast(i32)[:, :K, :], parm,
                                          zs_i[:, :K, 0:36])

            def emit_exps(b):
                # e = as + ad[p]; ex = exp(leaky(e)) = max(exp(e), exp(.2e));
                # the +ad folds into the Act bias (per-head calls)
                K = KBS[b]
                zsel = zsel_t[b]
                ex1 = ep.tile([P, KMAX, 4], f32, tag="ex1", name=f"ex1{b}")
                rhs = ep.tile([P, KMAX, 36], f32, tag="rhs", name=f"rhs{b}")
                ex1_t[b] = ex1
                rhs_t[b] = rhs
                rhs_bf = rhs[:].bitcast(mybir.dt.bfloat16)
                for hh in range(4):
                    nc.scalar.activation(ex1[:, :K, hh], zsel[:, :K, 32 + hh],
                                         mybir.ActivationFunctionType.Exp,
                                         bias=adst[:, b, hh:hh + 1])
                    nc.scalar.activation(rhs_bf[:, :K, 64 + hh], zsel[:, :K, 32 + hh],
                                         mybir.ActivationFunctionType.Exp,
                                         scale=ALPHA,
                                         bias=adst02[:, b, hh:hh + 1])

            def emit_rest(b):
                K = KBS[b]
                zsel = zsel_t.pop(b)
                ex1 = ex1_t.pop(b)
                rhs = rhs_t.pop(b)
                rhs_bf = rhs[:].bitcast(mybir.dt.bfloat16)
                nc.vector.tensor_tensor(out=rhs_bf[:, :K, 64:68],
                                        in0=rhs_bf[:, :K, 64:68],
                                        in1=ex1[:, :K, :], op=mybir.AluOpType.max)
                # num terms in bf16 occupy rhs words 0:32: t1 = z_sel * ex
                zsb = zsel[:].bitcast(mybir.dt.bfloat16)[:, :K, 0:64].rearrange(
                    "p k (h f) -> p k h f", h=4)
                exb = rhs_bf[:, :K, 64:68].unsqueeze(3).broadcast_to([P, K, 4, 16])
                t4 = rhs_bf[:, :K, 0:64].rearrange("p k (h f) -> p k h f", h=4)
                nc.vector.tensor_tensor(out=t4, in0=zsb, in1=exb,
                                        op=mybir.AluOpType.mult)
                # one fused reduce over chunks: [num x64 | ex x4] bf16 -> f32
                outd, bb = (outd1, b) if b < GSPLIT else (outd2, b - GSPLIT)
                nc.vector.tensor_reduce(
                    out=outd[:, bb, 0:68],
                    in_=rhs_bf[:, :K, 0:68].rearrange("p k f -> p f k"),
                    axis=mybir.AxisListType.X, op=mybir.AluOpType.add)

            recb = cpool.tile([P, BLOCKS, 4], f32)
            o3 = outO[:].rearrange("(p b) e -> p b e", p=P)

            def emit_norm(outd, b0, b1):
                # normalize + store blocks [b0, b1); outd holds exactly this
                # group so the dependency on its reduces is whole-tile
                nb = b1 - b0
                nc.vector.reciprocal(recb[:, b0:b1, :], outd[:, :, 64:68])
                rb = recb[:, b0:b1, :].unsqueeze(3).broadcast_to([P, nb, 4, 16])
                o4 = outb[:, 0:nb, :].rearrange("p b (h f) -> p b h f", h=4)
                nc.vector.tensor_tensor(
                    out=o4,
                    in0=outd[:, :, 0:64].rearrange("p b (h f) -> p b h f", h=4),
                    in1=rb, op=mybir.AluOpType.mult)
                nc.sync.dma_start(out=o3[:, b0:b1, :], in_=outb[:, 0:nb, :])

            LS, LE, LR = 1, 2, 3
            for i in range(BLOCKS + LR):
                if i < BLOCKS:
                    emit_gather(i)
                if 0 <= i - LS < BLOCKS:
                    emit_select(i - LS)
                if 0 <= i - LE < BLOCKS:
                    emit_exps(i - LE)
                if 0 <= i - LR < BLOCKS:
                    emit_rest(i - LR)
                if i - LR == GSPLIT - 1:
                    emit_norm(outd1, 0, GSPLIT)
            emit_norm(outd2, GSPLIT, BLOCKS)

    nc.finalize()
    return nc


def kernel(h, src, dst, W, A):
    from concourse.bass_utils import run_bass_kernel_spmd

    in_maps, KBS, _, unpack_maps = _build_host_data(h, src, dst, W, A)
    nc = _build_program(KBS)
    res = run_bass_kernel_spmd(nc, in_maps, core_ids=list(range(N_CORES)))
    out = np.empty((N_NODES, HF), dtype=np.float32)
    for k in range(N_CORES):
        out[k * NODES_PER_CORE:(k + 1) * NODES_PER_CORE] = \
            res.results[k]["outO"][unpack_maps[k]]
    return out


# revision 59
# speedup vs baseline: 1.0119x; 1.0004x over previous
"""GAT layer (nn_GATLayer) on 8 Trainium2 NeuronCores via Bass/Tile.

Strategy (dst-partitioned, diagonal-packed blocks, softmax fully local):
  - Core k owns dst nodes [k*6250, (k+1)*6250). All in-edges of those nodes are
    processed on core k, so segment-softmax needs no cross-core reduction.
  - Phase A (replicated): z = h @ W.T for ALL nodes with the attention
    projections folded in: rhs = [W.T | W.T@A1blk | W.T@A2blk] giving per-node
    rows [z(64) | as(4) | ad(4)] in one matmul (bf16 in, f32 PSUM). Node rows
    are packed PAIRWISE into 512B HBM rows: pair r = nodes (a, a+128) with
    a = (r//128)*256 + r%128, each half [z bf16 x64 | as f32 x4 | 112B pad].
    512B rows keep gather indices < 32768 (int16 SWDGE limit) with no
    small-transfer DMA penalty, at the cost of a per-edge half-select.
  - Phase A0: same matmul over own nodes only, in block-packed column order;
    ad(4) per own node stays SBUF-resident as adst[128 pos, 49 block, 4].
  - Edge phase: per core, dst nodes are sorted by in-degree (desc) and cut
    into 49 blocks of 128. Block b uses K_b = max in-degree in block chunks;
    slot (p, c) holds the c-th in-edge of the node at position p (pads point
    at a poison pair row whose both halves have as = -87 => exp ~ 0).
    Per block: dma_gather 512B pair rows by src; half-select the 144B node
    payload (Act copy of the hi half + DVE copy_predicated of the lo half);
    e = as + ad[p] folds into the Act exp bias (per-head calls);
    ex = max(exp(e), exp(0.2 e)) (leaky-relu folded, softmax shift-free);
    num = sum_c (z_sel * ex) via bf16 mult + f32 tensor_reduce; den likewise.
    Block emission is software-pipelined (gather/select/exps/rest staggered
    across iterations) so no engine queue head-of-line blocks another block's
    ready work. One final reciprocal + multiply normalizes all blocks; outO
    row = p*49 + b and the host permutes rows back to node order.
"""

import numpy as np

N_NODES = 50000
N_EDGES = 800000
IN_FEATS = 128
OUT_FEATS = 16
NUM_HEADS = 4
ALPHA = 0.2
HF = NUM_HEADS * OUT_FEATS  # 64

N_CORES = 8
P = 128
NODES_PER_CORE = N_NODES // N_CORES     # 6250
BLOCKS = 49                              # ceil(6250/128) blocks of 128 dst nodes
NODE_PAD = BLOCKS * P                    # 6272
N_NODES_PAD = 50176                      # 392 * 128 (node chunks in phase A)
NPAIR = N_NODES_PAD // 2                 # 25088 pair rows
POISON = NPAIR                           # poison pair row index
CMAX = 8                                 # chunks (1024 idxs) per dma_gather call


def _pair_row(n):
    return (n // 256) * 128 + (n % 128)


def _pair_half(n):
    return (n // 128) & 1


def _wrap16(vals):
    # stream position i -> idx tile [16, n/16] at [i%16, i//16]; rows
    # replicated to 128 partitions (SWDGE reads a [128, n/16] view).
    n = vals.shape[0]
    w = vals.reshape(n // 16, 16).T                    # [16, n/16]
    return np.ascontiguousarray(np.tile(w, (8, 1)))   # [128, n/16]


def _build_host_data(h, src, dst, W, A):
    """All index/layout prep. Returns per-core input dicts + block chunk counts."""
    src = np.asarray(src)
    dst = np.asarray(dst)
    W = np.asarray(W, dtype=np.float32)
    A = np.asarray(A, dtype=np.float32)
    h = np.asarray(h, dtype=np.float32)
    import ml_dtypes

    # folded weights: [W.T | W.T@A1blk | W.T@A2blk]  -> [128, 72]
    A1blk = np.zeros((HF, NUM_HEADS), dtype=np.float32)
    A2blk = np.zeros((HF, NUM_HEADS), dtype=np.float32)
    for hd in range(NUM_HEADS):
        A1blk[hd * OUT_FEATS:(hd + 1) * OUT_FEATS, hd] = A[hd, :OUT_FEATS]
        A2blk[hd * OUT_FEATS:(hd + 1) * OUT_FEATS, hd] = A[hd, OUT_FEATS:]
    WT = np.ascontiguousarray(W.T)                       # [128, 64]
    wcat = np.concatenate([WT, WT @ A1blk, WT @ A2blk], axis=1)  # [128, 72]
    wcat = wcat.astype(ml_dtypes.bfloat16)

    hT = np.zeros((P, N_NODES_PAD), dtype=ml_dtypes.bfloat16)    # [128, 50176]
    hT[:, :N_NODES] = h.T.astype(ml_dtypes.bfloat16)

    # poison pair row: both halves z=0, as=-87 -> ex ~ exp(-17.4) == ~0
    # but nonzero, so isolated nodes still get a finite denominator
    poison = np.zeros((1, 128), dtype=np.float32)
    poison[0, 32:36] = -87.0
    poison[0, 96:100] = -87.0

    # per-core edge runs grouped by dst
    order = np.argsort(dst, kind="stable")
    dst_s = dst[order]
    src_s = src[order]
    core_begin = np.searchsorted(dst_s, np.arange(0, N_NODES + 1, NODES_PER_CORE))

    # degree-sorted block assignment; K_b must be uniform across cores (SPMD)
    per_core = []
    kbs_cores = []
    for k in range(N_CORES):
        lo_e, hi_e = core_begin[k], core_begin[k + 1]
        cs = src_s[lo_e:hi_e]
        cd = dst_s[lo_e:hi_e] - k * NODES_PER_CORE      # local dst [0, 6250)
        deg = np.bincount(cd, minlength=NODES_PER_CORE)
        node_order = np.argsort(-deg, kind="stable")    # block = rank // 128
        degs = np.zeros(NODE_PAD, dtype=np.int64)
        degs[:NODES_PER_CORE] = deg[node_order]
        kbs = degs.reshape(BLOCKS, P).max(axis=1)
        kbs_cores.append(np.maximum(kbs, 1))
        # edge run start per local node (dst-sorted, stable)
        starts = np.zeros(NODES_PER_CORE + 1, dtype=np.int64)
        np.cumsum(deg, out=starts[1:])
        per_core.append((cs, node_order, deg, starts))

    KBS = tuple(int(x) for x in np.maximum.reduce(kbs_cores))
    SUMK = sum(KBS)
    S = P * SUMK                    # total edge slots per core
    offs = np.concatenate([[0], np.cumsum(KBS)]).astype(np.int64)

    in_maps = []
    unpack_maps = []
    for k in range(N_CORES):
        cs, node_order, deg, starts = per_core[k]
        # node -> (block, pos)
        blk = np.empty(NODES_PER_CORE, dtype=np.int64)
        pos = np.empty(NODES_PER_CORE, dtype=np.int64)
        ranks = np.empty(NODES_PER_CORE, dtype=np.int64)
        ranks[node_order] = np.arange(NODES_PER_CORE)
        blk = ranks // P
        pos = ranks % P
        unpack_maps.append(pos * BLOCKS + blk)          # node -> outO row

        gidx = np.full(S, POISON, dtype=np.int16)
        par = np.zeros((P, SUMK), dtype=np.int8)         # 1 -> take LO half
        cd = dst_s[core_begin[k]:core_begin[k + 1]] - k * NODES_PER_CORE
        ordinal = np.arange(len(cd)) - starts[cd]        # c-th in-edge of node
        offs_arr = offs[:-1]
        ci = offs_arr[blk[cd]] + ordinal                 # chunk column in tables
        gidx[ci * P + pos[cd]] = ((cs // 256) * 128 + (cs % 128)).astype(np.int16)
        par.reshape(-1)[pos[cd] * SUMK + ci] = ((cs // 128) & 1 == 0).astype(np.int8)

        # column b*128+pos = h of that node (block-packed own-node order)
        hT_own = np.zeros((P, NODE_PAD), dtype=ml_dtypes.bfloat16)
        cols = blk * P + pos
        hT_own[:, cols] = hT[:, np.arange(NODES_PER_CORE) + k * NODES_PER_CORE]

        in_maps.append({
            "hT": hT,
            "hT_own": hT_own,
            "wcat": wcat,
            "gidx": _wrap16(gidx),
            "par": np.ascontiguousarray(par),
            "poison": poison,
        })
    return in_maps, KBS, 0, unpack_maps


def _build_program(KBS, _unused=0):
    import concourse.bacc as bacc
    import concourse.tile as tile
    import concourse.mybir as mybir

    SUMK = sum(KBS)
    S = P * SUMK
    KMAX = max(KBS)
    offs = [0]
    for kb in KBS:
        offs.append(offs[-1] + kb)
    f32 = mybir.dt.float32
    bf16 = mybir.dt.bfloat16
    i16 = mybir.dt.int16
    i32 = mybir.dt.int32

    nc = bacc.Bacc("TRN2", target_bir_lowering=False, debug=False)

    hT = nc.dram_tensor("hT", [P, N_NODES_PAD], bf16, kind="ExternalInput")
    hT_own = nc.dram_tensor("hT_own", [P, NODE_PAD], bf16, kind="ExternalInput")
    wcat_d = nc.dram_tensor("wcat", [P, 72], bf16, kind="ExternalInput")
    gidx_d = nc.dram_tensor("gidx", [P, S // 16], i16, kind="ExternalInput")
    par_d = nc.dram_tensor("par", [P, SUMK], mybir.dt.int8, kind="ExternalInput")
    poison_d = nc.dram_tensor("poison", [1, 128], f32, kind="ExternalInput")

    zaug = nc.dram_tensor("zaug", [NPAIR + 1, 128], f32)  # 512B pair rows + poison
    outO = nc.dram_tensor("outO", [P * BLOCKS, HF], f32, kind="ExternalOutput")

    NCHUNK = N_NODES_PAD // P        # 392
    SC = 4                           # chunks per PSUM tile
    SC2 = 2 * SC                     # chunks per load/store super-chunk

    with tile.TileContext(nc) as tc:
        with (
            tc.tile_pool(name="const", bufs=1) as cpool,
            tc.tile_pool(name="pa", bufs=6) as pa,
            tc.tile_pool(name="papsum", bufs=4, space="PSUM") as papsum,
            tc.tile_pool(name="edge", bufs=4) as ep,
            tc.tile_pool(name="flush", bufs=3) as fp,
        ):
            wcat_t = cpool.tile([P, 72], bf16)
            nc.sync.dma_start(out=wcat_t[:], in_=wcat_d[:])
            gidx_t = cpool.tile([P, S // 16], i16)
            nc.sync.dma_start(out=gidx_t[:], in_=gidx_d[:])
            par_t = cpool.tile([P, SUMK], mybir.dt.int8)
            nc.sync.dma_start(out=par_t[:], in_=par_d[:])
            # poison row -> zaug[NPAIR]
            poi_t = cpool.tile([1, 128], f32)
            nc.sync.dma_start(out=poi_t[:], in_=poison_d[:])
            nc.sync.dma_start(out=zaug[NPAIR:NPAIR + 1, :], in_=poi_t[:])

            # ---------------- Phase A0: ad for own nodes (SBUF-resident) -----
            # (groups emitted interleaved into the phase A loop below)
            adst = cpool.tile([P, BLOCKS, 4], f32)
            hown_t = cpool.tile([P, NODE_PAD], bf16)
            nc.sync.dma_start(out=hown_t[:], in_=hT_own[:])

            def emit_a0_group(c0):
                c1 = min(c0 + SC, BLOCKS)
                zp = papsum.tile([P, SC, 72], f32, tag="zp")
                for j in range(c0, c1):
                    nc.tensor.matmul(
                        out=zp[:, j - c0, :],
                        lhsT=hown_t[:, j * P:(j + 1) * P],
                        rhs=wcat_t[:],
                        start=True, stop=True,
                    )
                nc.vector.tensor_copy(adst[:, c0:c1, :], zp[:, :c1 - c0, 68:72])

            # ---------------- Phase A: zaug pair rows for all nodes ----------
            n_sc = NCHUNK // SC2                        # 49, exact
            n_a0 = (BLOCKS + SC - 1) // SC              # 13 A0 groups
            z3 = zaug[0:NPAIR, :].rearrange("(c2 q) (c01 e) -> q c2 c01 e",
                                            q=P, c01=2)
            for sidx in range(n_sc):
                if sidx < n_a0:
                    emit_a0_group(sidx * SC)
                base = sidx * SC2 * P
                hsl = pa.tile([P, SC2 * P], bf16, tag="hsl")
                nc.sync.dma_start(out=hsl[:], in_=hT[:, base:base + SC2 * P])
                zst = pa.tile([P, SC2, 64], f32, tag="zst")
                zbf = zst[:].bitcast(mybir.dt.bfloat16)
                for g0 in range(0, SC2, SC):
                    g1 = g0 + SC
                    zp = papsum.tile([P, SC, 72], f32, tag="zp")
                    for j in range(g0, g1):
                        nc.tensor.matmul(
                            out=zp[:, j - g0, :],
                            lhsT=hsl[:, j * P:(j + 1) * P],
                            rhs=wcat_t[:],
                            start=True, stop=True,
                        )
                    nc.scalar.copy(out=zbf[:, g0:g1, 0:64], in_=zp[:, :SC, 0:64])
                    nc.vector.tensor_copy(zst[:, g0:g1, 32:36], zp[:, :SC, 64:68])
                zs4 = zst[:].rearrange("p (c2 c01) e -> p c2 c01 e", c01=2)
                nc.sync.dma_start(
                    out=z3[:, sidx * SC2 // 2:(sidx + 1) * SC2 // 2, :, :],
                    in_=zs4[:],
                )
            adst02 = cpool.tile([P, BLOCKS, 4], f32)
            nc.vector.tensor_scalar(out=adst02[:], in0=adst[:], scalar1=ALPHA,
                                    scalar2=None, op0=mybir.AluOpType.mult)

            # ---------------- Edge phase ----------------
            # Software-pipelined issue order: emitting gather(b), select(b-LS),
            # exps(b-LE), rest(b-LR) per iteration keeps every engine queue
            # free of head-of-line blocking (in-order queues would otherwise
            # serialize each block's Act<->DVE ping-pong).
            # [num(64) | den(4)] per block, split into two tiles so the first
            # group's normalization (whole-tile dependency) can issue while
            # later blocks still compute
            GSPLIT = 44
            outd1 = cpool.tile([P, GSPLIT, 68], f32)
            outd2 = cpool.tile([P, BLOCKS - GSPLIT, 68], f32)
            outb = cpool.tile([P, GSPLIT, HF], f32)   # reused by both norm groups
            zs_t = {}
            zsel_t = {}
            ex1_t = {}
            rhs_t = {}

            def emit_gather(b):
                K = KBS[b]
                off = offs[b]
                zs = ep.tile([P, KMAX, 128], f32, tag="zs", name=f"zs{b}")
                zs_t[b] = zs
                for c0 in range(0, K, CMAX):
                    c1 = min(c0 + CMAX, K)
                    nc.gpsimd.dma_gather(
                        out_ap=zs[:, c0:c1, :],
                        in_ap=zaug[:],
                        idxs_ap=gidx_t[:, (off + c0) * 8:(off + c1) * 8],
                        num_idxs=(c1 - c0) * P,
                        num_idxs_reg=(c1 - c0) * P,
                        elem_size=128,
                    )

            def emit_select(b):
                # half-select the 144B payload [z bf16 x64 | as f32 x4]:
                # hi-half copied on Act (z words are never NaN/denormal here),
                # then lo words overwrite where par!=0 (DVE predicated copy)
                K = KBS[b]
                off = offs[b]
                zs = zs_t.pop(b)
                zs_i = zs[:].bitcast(i32)
                zsel = ep.tile([P, KMAX, 36], f32, tag="zsel", name=f"zsel{b}")
                zsel_t[b] = zsel
                nc.scalar.copy(out=zsel[:, :K, :], in_=zs[:, :K, 64:100])
                parm = par_t[:, off:off + K].unsqueeze(2).broadcast_to([P, K, 36])
                nc.vector.copy_predicated(zsel[:].bitcast(i32)[:, :K, :], parm,
                                          zs_i[:, :K, 0:36])

            def emit_exps(b):
                # e = as + ad[p]; ex = exp(leaky(e)) = max(exp(e), exp(.2e));
                # the +ad folds into the Act bias (per-head calls)
                K = KBS[b]
                zsel = zsel_t[b]
                ex1 = ep.tile([P, KMAX, 4], f32, tag="ex1", name=f"ex1{b}")
                rhs = ep.tile([P, KMAX, 36], f32, tag="rhs", name=f"rhs{b}")
                ex1_t[b] = ex1
                rhs_t[b] = rhs
                rhs_bf = rhs[:].bitcast(mybir.dt.bfloat16)
                for hh in range(4):
                    nc.scalar.activation(ex1[:, :K, hh], zsel[:, :K, 32 + hh],
                                         mybir.ActivationFunctionType.Exp,
                                         bias=adst[:, b, hh:hh + 1])
                    nc.scalar.activation(rhs_bf[:, :K, 64 + hh], zsel[:, :K, 32 + hh],
                                         mybir.ActivationFunctionType.Exp,
                                         scale=ALPHA,
                                         bias=adst02[:, b, hh:hh + 1])

            def emit_rest(b):
                K = KBS[b]
                zsel = zsel_t.pop(b)
                ex1 = ex1_t.pop(b)
                rhs = rhs_t.pop(b)
                rhs_bf = rhs[:].bitcast(mybir.dt.bfloat16)
                nc.vector.tensor_tensor(out=rhs_bf[:, :K, 64:68],
                                        in0=rhs_bf[:, :K, 64:68],
                                        in1=ex1[:, :K, :], op=mybir.AluOpType.max)
                # num terms in bf16 occupy rhs words 0:32: t1 = z_sel * ex
                zsb = zsel[:].bitcast(mybir.dt.bfloat16)[:, :K, 0:64].rearrange(
                    "p k (h f) -> p k h f", h=4)
                exb = rhs_bf[:, :K, 64:68].unsqueeze(3).broadcast_to([P, K, 4, 16])
                t4 = rhs_bf[:, :K, 0:64].rearrange("p k (h f) -> p k h f", h=4)
                nc.vector.tensor_tensor(out=t4, in0=zsb, in1=exb,
                                        op=mybir.AluOpType.mult)
                # one fused reduce over chunks: [num x64 | ex x4] bf16 -> f32
                outd, bb = (outd1, b) if b < GSPLIT else (outd2, b - GSPLIT)
                nc.vector.tensor_reduce(
                    out=outd[:, bb, 0:68],
                    in_=rhs_bf[:, :K, 0:68].rearrange("p k f -> p f k"),
                    axis=mybir.AxisListType.X, op=mybir.AluOpType.add)

            recb = cpool.tile([P, BLOCKS, 4], f32)
            o3 = outO[:].rearrange("(p b) e -> p b e", p=P)

            def emit_norm(outd, b0, b1):
                # normalize + store blocks [b0, b1); outd holds exactly this
                # group so the dependency on its reduces is whole-tile
                nb = b1 - b0
                nc.vector.reciprocal(recb[:, b0:b1, :], outd[:, :, 64:68])
                rb = recb[:, b0:b1, :].unsqueeze(3).broadcast_to([P, nb, 4, 16])
                o4 = outb[:, 0:nb, :].rearrange("p b (h f) -> p b h f", h=4)
                nc.vector.tensor_tensor(
                    out=o4,
                    in0=outd[:, :, 0:64].rearrange("p b (h f) -> p b h f", h=4),
                    in1=rb, op=mybir.AluOpType.mult)
                nc.sync.dma_start(out=o3[:, b0:b1, :], in_=outb[:, 0:nb, :])

            LS, LE, LR = 1, 2, 3
            for i in range(BLOCKS + LR):
                if i < BLOCKS:
                    emit_gather(i)
                if 0 <= i - LS < BLOCKS:
                    emit_select(i - LS)
                if 0 <= i - LE < BLOCKS:
                    emit_exps(i - LE)
                if 0 <= i - LR < BLOCKS:
                    emit_rest(i - LR)
                if i - LR == GSPLIT - 1:
                    emit_norm(outd1, 0, GSPLIT)
            emit_norm(outd2, GSPLIT, BLOCKS)

    nc.finalize()
    return nc


def kernel(h, src, dst, W, A):
    from concourse.bass_utils import run_bass_kernel_spmd

    in_maps, KBS, _, unpack_maps = _build_host_data(h, src, dst, W, A)
    nc = _build_program(KBS)
    res = run_bass_kernel_spmd(nc, in_maps, core_ids=list(range(N_CORES)))
    out = np.empty((N_NODES, HF), dtype=np.float32)
    for k in range(N_CORES):
        out[k * NODES_PER_CORE:(k + 1) * NODES_PER_CORE] = \
            res.results[k]["outO"][unpack_maps[k]]
    return out


# revision 60
# speedup vs baseline: 1.0122x; 1.0003x over previous
"""GAT layer (nn_GATLayer) on 8 Trainium2 NeuronCores via Bass/Tile.

Strategy (dst-partitioned, diagonal-packed blocks, softmax fully local):
  - Core k owns dst nodes [k*6250, (k+1)*6250). All in-edges of those nodes are
    processed on core k, so segment-softmax needs no cross-core reduction.
  - Phase A (replicated): z = h @ W.T for ALL nodes with the attention
    projections folded in: rhs = [W.T | W.T@A1blk | W.T@A2blk] giving per-node
    rows [z(64) | as(4) | ad(4)] in one matmul (bf16 in, f32 PSUM). Node rows
    are packed PAIRWISE into 512B HBM rows: pair r = nodes (a, a+128) with
    a = (r//128)*256 + r%128, each half [z bf16 x64 | as f32 x4 | 112B pad].
    512B rows keep gather indices < 32768 (int16 SWDGE limit) with no
    small-transfer DMA penalty, at the cost of a per-edge half-select.
  - Phase A0: same matmul over own nodes only, in block-packed column order;
    ad(4) per own node stays SBUF-resident as adst[128 pos, 49 block, 4].
  - Edge phase: per core, dst nodes are sorted by in-degree (desc) and cut
    into 49 blocks of 128. Block b uses K_b = max in-degree in block chunks;
    slot (p, c) holds the c-th in-edge of the node at position p (pads point
    at a poison pair row whose both halves have as = -87 => exp ~ 0).
    Per block: dma_gather 512B pair rows by src; half-select the 144B node
    payload (Act copy of the hi half + DVE copy_predicated of the lo half);
    e = as + ad[p] folds into the Act exp bias (per-head calls);
    ex = max(exp(e), exp(0.2 e)) (leaky-relu folded, softmax shift-free);
    num = sum_c (z_sel * ex) via bf16 mult + f32 tensor_reduce; den likewise.
    Block emission is software-pipelined (gather/select/exps/rest staggered
    across iterations) so no engine queue head-of-line blocks another block's
    ready work. One final reciprocal + multiply normalizes all blocks; outO
    row = p*49 + b and the host permutes rows back to node order.
"""

import numpy as np

N_NODES = 50000
N_EDGES = 800000
IN_FEATS = 128
OUT_FEATS = 16
NUM_HEADS = 4
ALPHA = 0.2
HF = NUM_HEADS * OUT_FEATS  # 64

N_CORES = 8
P = 128
NODES_PER_CORE = N_NODES // N_CORES     # 6250
BLOCKS = 49                              # ceil(6250/128) blocks of 128 dst nodes
NODE_PAD = BLOCKS * P                    # 6272
N_NODES_PAD = 50176                      # 392 * 128 (node chunks in phase A)
NPAIR = N_NODES_PAD // 2                 # 25088 pair rows
POISON = NPAIR                           # poison pair row index
CMAX = 8                                 # chunks (1024 idxs) per dma_gather call


def _pair_row(n):
    return (n // 256) * 128 + (n % 128)


def _pair_half(n):
    return (n // 128) & 1


def _wrap16(vals):
    # stream position i -> idx tile [16, n/16] at [i%16, i//16]; rows
    # replicated to 128 partitions (SWDGE reads a [128, n/16] view).
    n = vals.shape[0]
    w = vals.reshape(n // 16, 16).T                    # [16, n/16]
    return np.ascontiguousarray(np.tile(w, (8, 1)))   # [128, n/16]


def _build_host_data(h, src, dst, W, A):
    """All index/layout prep. Returns per-core input dicts + block chunk counts."""
    src = np.asarray(src)
    dst = np.asarray(dst)
    W = np.asarray(W, dtype=np.float32)
    A = np.asarray(A, dtype=np.float32)
    h = np.asarray(h, dtype=np.float32)
    import ml_dtypes

    # folded weights: [W.T | W.T@A1blk | W.T@A2blk]  -> [128, 72]
    A1blk = np.zeros((HF, NUM_HEADS), dtype=np.float32)
    A2blk = np.zeros((HF, NUM_HEADS), dtype=np.float32)
    for hd in range(NUM_HEADS):
        A1blk[hd * OUT_FEATS:(hd + 1) * OUT_FEATS, hd] = A[hd, :OUT_FEATS]
        A2blk[hd * OUT_FEATS:(hd + 1) * OUT_FEATS, hd] = A[hd, OUT_FEATS:]
    WT = np.ascontiguousarray(W.T)                       # [128, 64]
    wcat = np.concatenate([WT, WT @ A1blk, WT @ A2blk], axis=1)  # [128, 72]
    wcat = wcat.astype(ml_dtypes.bfloat16)

    hT = np.zeros((P, N_NODES_PAD), dtype=ml_dtypes.bfloat16)    # [128, 50176]
    hT[:, :N_NODES] = h.T.astype(ml_dtypes.bfloat16)

    # poison pair row: both halves z=0, as=-87 -> ex ~ exp(-17.4) == ~0
    # but nonzero, so isolated nodes still get a finite denominator
    poison = np.zeros((1, 128), dtype=np.float32)
    poison[0, 32:36] = -87.0
    poison[0, 96:100] = -87.0

    # per-core edge runs grouped by dst
    order = np.argsort(dst, kind="stable")
    dst_s = dst[order]
    src_s = src[order]
    core_begin = np.searchsorted(dst_s, np.arange(0, N_NODES + 1, NODES_PER_CORE))

    # degree-sorted block assignment; K_b must be uniform across cores (SPMD)
    per_core = []
    kbs_cores = []
    for k in range(N_CORES):
        lo_e, hi_e = core_begin[k], core_begin[k + 1]
        cs = src_s[lo_e:hi_e]
        cd = dst_s[lo_e:hi_e] - k * NODES_PER_CORE      # local dst [0, 6250)
        deg = np.bincount(cd, minlength=NODES_PER_CORE)
        node_order = np.argsort(-deg, kind="stable")    # block = rank // 128
        degs = np.zeros(NODE_PAD, dtype=np.int64)
        degs[:NODES_PER_CORE] = deg[node_order]
        kbs = degs.reshape(BLOCKS, P).max(axis=1)
        kbs_cores.append(np.maximum(kbs, 1))
        # edge run start per local node (dst-sorted, stable)
        starts = np.zeros(NODES_PER_CORE + 1, dtype=np.int64)
        np.cumsum(deg, out=starts[1:])
        per_core.append((cs, node_order, deg, starts))

    KBS = tuple(int(x) for x in np.maximum.reduce(kbs_cores))
    SUMK = sum(KBS)
    S = P * SUMK                    # total edge slots per core
    offs = np.concatenate([[0], np.cumsum(KBS)]).astype(np.int64)

    in_maps = []
    unpack_maps = []
    for k in range(N_CORES):
        cs, node_order, deg, starts = per_core[k]
        # node -> (block, pos)
        blk = np.empty(NODES_PER_CORE, dtype=np.int64)
        pos = np.empty(NODES_PER_CORE, dtype=np.int64)
        ranks = np.empty(NODES_PER_CORE, dtype=np.int64)
        ranks[node_order] = np.arange(NODES_PER_CORE)
        blk = ranks // P
        pos = ranks % P
        unpack_maps.append(pos * BLOCKS + blk)          # node -> outO row

        gidx = np.full(S, POISON, dtype=np.int16)
        par = np.zeros((P, SUMK), dtype=np.int8)         # 1 -> take LO half
        cd = dst_s[core_begin[k]:core_begin[k + 1]] - k * NODES_PER_CORE
        ordinal = np.arange(len(cd)) - starts[cd]        # c-th in-edge of node
        offs_arr = offs[:-1]
        ci = offs_arr[blk[cd]] + ordinal                 # chunk column in tables
        gidx[ci * P + pos[cd]] = ((cs // 256) * 128 + (cs % 128)).astype(np.int16)
        par.reshape(-1)[pos[cd] * SUMK + ci] = ((cs // 128) & 1 == 0).astype(np.int8)

        # column b*128+pos = h of that node (block-packed own-node order)
        hT_own = np.zeros((P, NODE_PAD), dtype=ml_dtypes.bfloat16)
        cols = blk * P + pos
        hT_own[:, cols] = hT[:, np.arange(NODES_PER_CORE) + k * NODES_PER_CORE]

        in_maps.append({
            "hT": hT,
            "hT_own": hT_own,
            "wcat": wcat,
            "gidx": _wrap16(gidx),
            "par": np.ascontiguousarray(par),
            "poison": poison,
        })
    return in_maps, KBS, 0, unpack_maps


def _build_program(KBS, _unused=0):
    import concourse.bacc as bacc
    import concourse.tile as tile
    import concourse.mybir as mybir

    SUMK = sum(KBS)
    S = P * SUMK
    KMAX = max(KBS)
    offs = [0]
    for kb in KBS:
        offs.append(offs[-1] + kb)
    f32 = mybir.dt.float32
    bf16 = mybir.dt.bfloat16
    i16 = mybir.dt.int16
    i32 = mybir.dt.int32

    nc = bacc.Bacc("TRN2", target_bir_lowering=False, debug=False)

    hT = nc.dram_tensor("hT", [P, N_NODES_PAD], bf16, kind="ExternalInput")
    hT_own = nc.dram_tensor("hT_own", [P, NODE_PAD], bf16, kind="ExternalInput")
    wcat_d = nc.dram_tensor("wcat", [P, 72], bf16, kind="ExternalInput")
    gidx_d = nc.dram_tensor("gidx", [P, S // 16], i16, kind="ExternalInput")
    par_d = nc.dram_tensor("par", [P, SUMK], mybir.dt.int8, kind="ExternalInput")
    poison_d = nc.dram_tensor("poison", [1, 128], f32, kind="ExternalInput")

    zaug = nc.dram_tensor("zaug", [NPAIR + 1, 128], f32)  # 512B pair rows + poison
    outO = nc.dram_tensor("outO", [P * BLOCKS, HF], f32, kind="ExternalOutput")

    NCHUNK = N_NODES_PAD // P        # 392
    SC = 4                           # chunks per PSUM tile
    SC2 = 2 * SC                     # chunks per load/store super-chunk

    with tile.TileContext(nc) as tc:
        with (
            tc.tile_pool(name="const", bufs=1) as cpool,
            tc.tile_pool(name="pa", bufs=8) as pa,
            tc.tile_pool(name="papsum", bufs=4, space="PSUM") as papsum,
            tc.tile_pool(name="edge", bufs=4) as ep,
            tc.tile_pool(name="flush", bufs=3) as fp,
        ):
            wcat_t = cpool.tile([P, 72], bf16)
            nc.sync.dma_start(out=wcat_t[:], in_=wcat_d[:])
            gidx_t = cpool.tile([P, S // 16], i16)
            nc.sync.dma_start(out=gidx_t[:], in_=gidx_d[:])
            par_t = cpool.tile([P, SUMK], mybir.dt.int8)
            nc.sync.dma_start(out=par_t[:], in_=par_d[:])
            # poison row -> zaug[NPAIR]
            poi_t = cpool.tile([1, 128], f32)
            nc.sync.dma_start(out=poi_t[:], in_=poison_d[:])
            nc.sync.dma_start(out=zaug[NPAIR:NPAIR + 1, :], in_=poi_t[:])

            # ---------------- Phase A0: ad for own nodes (SBUF-resident) -----
            # (groups emitted interleaved into the phase A loop below)
            adst = cpool.tile([P, BLOCKS, 4], f32)
            hown_t = cpool.tile([P, NODE_PAD], bf16)
            nc.sync.dma_start(out=hown_t[:], in_=hT_own[:])

            def emit_a0_group(c0):
                c1 = min(c0 + SC, BLOCKS)
                zp = papsum.tile([P, SC, 72], f32, tag="zp")
                for j in range(c0, c1):
                    nc.tensor.matmul(
                        out=zp[:, j - c0, :],
                        lhsT=hown_t[:, j * P:(j + 1) * P],
                        rhs=wcat_t[:],
                        start=True, stop=True,
                    )
                nc.vector.tensor_copy(adst[:, c0:c1, :], zp[:, :c1 - c0, 68:72])

            # ---------------- Phase A: zaug pair rows for all nodes ----------
            n_sc = NCHUNK // SC2                        # 49, exact
            n_a0 = (BLOCKS + SC - 1) // SC              # 13 A0 groups
            z3 = zaug[0:NPAIR, :].rearrange("(c2 q) (c01 e) -> q c2 c01 e",
                                            q=P, c01=2)
            for sidx in range(n_sc):
                if sidx < n_a0:
                    emit_a0_group(sidx * SC)
                base = sidx * SC2 * P
                hsl = pa.tile([P, SC2 * P], bf16, tag="hsl")
                nc.sync.dma_start(out=hsl[:], in_=hT[:, base:base + SC2 * P])
                zst = pa.tile([P, SC2, 64], f32, tag="zst")
                zbf = zst[:].bitcast(mybir.dt.bfloat16)
                for g0 in range(0, SC2, SC):
                    g1 = g0 + SC
                    zp = papsum.tile([P, SC, 72], f32, tag="zp")
                    for j in range(g0, g1):
                        nc.tensor.matmul(
                            out=zp[:, j - g0, :],
                            lhsT=hsl[:, j * P:(j + 1) * P],
                            rhs=wcat_t[:],
                            start=True, stop=True,
                        )
                    nc.scalar.copy(out=zbf[:, g0:g1, 0:64], in_=zp[:, :SC, 0:64])
                    nc.vector.tensor_copy(zst[:, g0:g1, 32:36], zp[:, :SC, 64:68])
                zs4 = zst[:].rearrange("p (c2 c01) e -> p c2 c01 e", c01=2)
                nc.sync.dma_start(
                    out=z3[:, sidx * SC2 // 2:(sidx + 1) * SC2 // 2, :, :],
                    in_=zs4[:],
                )
            adst02 = cpool.tile([P, BLOCKS, 4], f32)
            nc.vector.tensor_scalar(out=adst02[:], in0=adst[:], scalar1=ALPHA,
                                    scalar2=None, op0=mybir.AluOpType.mult)

            # ---------------- Edge phase ----------------
            # Software-pipelined issue order: emitting gather(b), select(b-LS),
            # exps(b-LE), rest(b-LR) per iteration keeps every engine queue
            # free of head-of-line blocking (in-order queues would otherwise
            # serialize each block's Act<->DVE ping-pong).
            # [num(64) | den(4)] per block, split into two tiles so the first
            # group's normalization (whole-tile dependency) can issue while
            # later blocks still compute
            GSPLIT = 44
            outd1 = cpool.tile([P, GSPLIT, 68], f32)
            outd2 = cpool.tile([P, BLOCKS - GSPLIT, 68], f32)
            outb = cpool.tile([P, GSPLIT, HF], f32)   # reused by both norm groups
            zs_t = {}
            zsel_t = {}
            ex1_t = {}
            rhs_t = {}

            def emit_gather(b):
                K = KBS[b]
                off = offs[b]
                zs = ep.tile([P, KMAX, 128], f32, tag="zs", name=f"zs{b}")
                zs_t[b] = zs
                for c0 in range(0, K, CMAX):
                    c1 = min(c0 + CMAX, K)
                    nc.gpsimd.dma_gather(
                        out_ap=zs[:, c0:c1, :],
                        in_ap=zaug[:],
                        idxs_ap=gidx_t[:, (off + c0) * 8:(off + c1) * 8],
                        num_idxs=(c1 - c0) * P,
                        num_idxs_reg=(c1 - c0) * P,
                        elem_size=128,
                    )

            def emit_select(b):
                # half-select the 144B payload [z bf16 x64 | as f32 x4]:
                # hi-half copied on Act (z words are never NaN/denormal here),
                # then lo words overwrite where par!=0 (DVE predicated copy)
                K = KBS[b]
                off = offs[b]
                zs = zs_t.pop(b)
                zs_i = zs[:].bitcast(i32)
                zsel = ep.tile([P, KMAX, 36], f32, tag="zsel", name=f"zsel{b}")
                zsel_t[b] = zsel
                nc.scalar.copy(out=zsel[:, :K, :], in_=zs[:, :K, 64:100])
                parm = par_t[:, off:off + K].unsqueeze(2).broadcast_to([P, K, 36])
                nc.vector.copy_predicated(zsel[:].bitcast(i32)[:, :K, :], parm,
                                          zs_i[:, :K, 0:36])

            def emit_exps(b):
                # e = as + ad[p]; ex = exp(leaky(e)) = max(exp(e), exp(.2e));
                # the +ad folds into the Act bias (per-head calls)
                K = KBS[b]
                zsel = zsel_t[b]
                ex1 = ep.tile([P, KMAX, 4], f32, tag="ex1", name=f"ex1{b}")
                rhs = ep.tile([P, KMAX, 36], f32, tag="rhs", name=f"rhs{b}")
                ex1_t[b] = ex1
                rhs_t[b] = rhs
                rhs_bf = rhs[:].bitcast(mybir.dt.bfloat16)
                for hh in range(4):
                    nc.scalar.activation(ex1[:, :K, hh], zsel[:, :K, 32 + hh],
                                         mybir.ActivationFunctionType.Exp,
                                         bias=adst[:, b, hh:hh + 1])
                    nc.scalar.activation(rhs_bf[:, :K, 64 + hh], zsel[:, :K, 32 + hh],
                                         mybir.ActivationFunctionType.Exp,
                                         scale=ALPHA,
                                         bias=adst02[:, b, hh:hh + 1])

            def emit_rest(b):
                K = KBS[b]
                zsel = zsel_t.pop(b)
                ex1 = ex1_t.pop(b)
                rhs = rhs_t.pop(b)
                rhs_bf = rhs[:].bitcast(mybir.dt.bfloat16)
                nc.vector.tensor_tensor(out=rhs_bf[:, :K, 64:68],
                                        in0=rhs_bf[:, :K, 64:68],
                                        in1=ex1[:, :K, :], op=mybir.AluOpType.max)
                # num terms in bf16 occupy rhs words 0:32: t1 = z_sel * ex
                zsb = zsel[:].bitcast(mybir.dt.bfloat16)[:, :K, 0:64].rearrange(
                    "p k (h f) -> p k h f", h=4)
                exb = rhs_bf[:, :K, 64:68].unsqueeze(3).broadcast_to([P, K, 4, 16])
                t4 = rhs_bf[:, :K, 0:64].rearrange("p k (h f) -> p k h f", h=4)
                nc.vector.tensor_tensor(out=t4, in0=zsb, in1=exb,
                                        op=mybir.AluOpType.mult)
                # one fused reduce over chunks: [num x64 | ex x4] bf16 -> f32
                outd, bb = (outd1, b) if b < GSPLIT else (outd2, b - GSPLIT)
                nc.vector.tensor_reduce(
                    out=outd[:, bb, 0:68],
                    in_=rhs_bf[:, :K, 0:68].rearrange("p k f -> p f k"),
                    axis=mybir.AxisListType.X, op=mybir.AluOpType.add)

            recb = cpool.tile([P, BLOCKS, 4], f32)
            o3 = outO[:].rearrange("(p b) e -> p b e", p=P)

            def emit_norm(outd, b0, b1):
                # normalize + store blocks [b0, b1); outd holds exactly this
                # group so the dependency on its reduces is whole-tile
                nb = b1 - b0
                nc.vector.reciprocal(recb[:, b0:b1, :], outd[:, :, 64:68])
                rb = recb[:, b0:b1, :].unsqueeze(3).broadcast_to([P, nb, 4, 16])
                o4 = outb[:, 0:nb, :].rearrange("p b (h f) -> p b h f", h=4)
                nc.vector.tensor_tensor(
                    out=o4,
                    in0=outd[:, :, 0:64].rearrange("p b (h f) -> p b h f", h=4),
                    in1=rb, op=mybir.AluOpType.mult)
                nc.sync.dma_start(out=o3[:, b0:b1, :], in_=outb[:, 0:nb, :])

            LS, LE, LR = 1, 2, 3
            for i in range(BLOCKS + LR):
                if i < BLOCKS:
                    emit_gather(i)
                if 0 <= i - LS < BLOCKS:
                    emit_select(i - LS)
                if 0 <= i - LE < BLOCKS:
                    emit_exps(i - LE)
                if 0 <= i - LR < BLOCKS:
                    emit_rest(i - LR)
                if i - LR == GSPLIT - 1:
                    emit_norm(outd1, 0, GSPLIT)
            emit_norm(outd2, GSPLIT, BLOCKS)

    nc.finalize()
    return nc


def kernel(h, src, dst, W, A):
    from concourse.bass_utils import run_bass_kernel_spmd

    in_maps, KBS, _, unpack_maps = _build_host_data(h, src, dst, W, A)
    nc = _build_program(KBS)
    res = run_bass_kernel_spmd(nc, in_maps, core_ids=list(range(N_CORES)))
    out = np.empty((N_NODES, HF), dtype=np.float32)
    for k in range(N_CORES):
        out[k * NODES_PER_CORE:(k + 1) * NODES_PER_CORE] = \
            res.results[k]["outO"][unpack_maps[k]]
    return out
